# revision 1
# baseline (speedup 1.0000x reference)
"""GNN IntraAgg kernel for Trainium2 (8 NeuronCores, SPMD data-parallel).

Computation (per node b):
    feats_1[b] = mean_k embedding[neighbor_idx[b, k]]      # [D]
    feats_2[b] = self_feats[b] - feats_1[b]                # [D]
    out[b]     = concat(feats_1[b], feats_2[b])            # [2D]

Sharding: batch axis split 8 ways (6250 nodes/core, padded to 6272 = 49*128);
embedding table replicated per core.

HW note: one indirect DMA consumes ONE offset per destination partition, so
each gather instruction fetches 128 table rows = 4 nodes x 32 neighbors
(slot-per-partition layout). The K-axis mean is a partition-axis reduction,
done on the TensorEngine: 32 accumulating matmuls per 128-node group against
constant 1/32 block-diagonal masks (lhsT[s, n] = 1/32 iff slot s belongs to
node n), leaving feats_1 for 128 nodes in one PSUM tile.

Host-side marshalling: neighbor_idx is transposed to [128, G*32] so that
column i of the SBUF index tile holds the 128 flat (node, k) slots of gather
instruction i; the masks are a compile-time constant shipped as an input.
"""

import numpy as np

N_EMBED, D = 200000, 128
B, K = 50000, 32
N_CORES = 8
P = 128
B_LOCAL = B // N_CORES            # 6250
G = (B_LOCAL + P - 1) // P        # 49 groups of 128 nodes
B_PAD = G * P                     # 6272
NPI = P // K                      # 4 nodes per gather instruction
JPG = P // NPI                    # 32 gather instructions per group
NI = G * JPG                      # 1568 gather instructions total

_cache: dict = {}


def make_masks() -> np.ndarray:
    """masks_t[s, j*128 + n] = 1/K iff n == 4*j + s//K  (lhsT layout)."""
    masks = np.zeros((JPG, P, P), np.float32)
    j = np.arange(JPG)[:, None]
    s = np.arange(P)[None, :]
    n = NPI * j + s // K                      # [JPG, P]
    masks[j, s, n] = 1.0 / K
    return np.ascontiguousarray(masks.transpose(1, 0, 2).reshape(P, JPG * P))


def build_bass(gather_bufs: int = 24):
    import concourse.bass as bass
    import concourse.mybir as mybir
    import concourse.tile as tile
    from concourse import bacc

    nc = bacc.Bacc(
        "TRN2",
        target_bir_lowering=False,
        debug=False,
        enable_asserts=True,
        num_devices=N_CORES,
    )
    emb = nc.dram_tensor(
        "embedding", [N_EMBED, D], mybir.dt.float32, kind="ExternalInput"
    ).ap()
    sf = nc.dram_tensor(
        "self_feats", [B_PAD, D], mybir.dt.float32, kind="ExternalInput"
    ).ap()
    nit = nc.dram_tensor(
        "neighbor_idx_t", [P, NI], mybir.dt.int32, kind="ExternalInput"
    ).ap()
    masks = nc.dram_tensor(
        "masks", [P, JPG * P], mybir.dt.float32, kind="ExternalInput"
    ).ap()
    out = nc.dram_tensor(
        "out", [B_PAD, 2 * D], mybir.dt.float32, kind="ExternalOutput"
    ).ap()

    with tile.TileContext(nc) as tc:
        with (
            tc.tile_pool(name="const", bufs=1) as const_tp,
            tc.tile_pool(name="gather", bufs=gather_bufs) as gather_tp,
            tc.tile_pool(name="psum", bufs=4, space="PSUM") as psum_tp,
            tc.tile_pool(name="io", bufs=6) as io_tp,
        ):
            idx_sb = const_tp.tile([P, NI], mybir.dt.int32, tag="idx")
            nc.sync.dma_start(out=idx_sb[:], in_=nit[:, :])
            mask_sb = const_tp.tile([P, JPG * P], mybir.dt.float32, tag="mask")
            nc.sync.dma_start(out=mask_sb[:], in_=masks[:, :])

            for g in range(G):
                r0 = g * P
                self_t = io_tp.tile([P, D], mybir.dt.float32, tag="self")
                nc.sync.dma_start(out=self_t[:], in_=sf[r0 : r0 + P, :])

                ps = psum_tp.tile([P, D], mybir.dt.float32, tag="ps")
                for j in range(JPG):
                    i = g * JPG + j
                    gt = gather_tp.tile([P, D], mybir.dt.float32, tag="g")
                    nc.gpsimd.indirect_dma_start(
                        out=gt[:],
                        out_offset=None,
                        in_=emb[:, :],
                        in_offset=bass.IndirectOffsetOnAxis(
                            ap=idx_sb[:, i : i + 1], axis=0
                        ),
                    )
                    nc.tensor.matmul(
                        out=ps[:],
                        lhsT=mask_sb[:, j * P : (j + 1) * P],
                        rhs=gt[:],
                        start=(j == 0),
                        stop=(j == JPG - 1),
                    )

                out_t = io_tp.tile([P, 2 * D], mybir.dt.float32, tag="out")
                nc.vector.tensor_copy(out=out_t[:, :D], in_=ps[:])
                nc.vector.tensor_tensor(
                    out=out_t[:, D:],
                    in0=self_t[:],
                    in1=ps[:],
                    op=mybir.AluOpType.subtract,
                )
                nc.sync.dma_start(out=out[r0 : r0 + P, :], in_=out_t[:])

    nc.compile()
    return nc


def make_in_maps(embedding, self_feats, neighbor_idx):
    embedding = np.ascontiguousarray(embedding, dtype=np.float32)
    sf = np.asarray(self_feats, dtype=np.float32).reshape(N_CORES, B_LOCAL, D)
    ni = np.asarray(neighbor_idx, dtype=np.int32).reshape(N_CORES, B_LOCAL, K)
    sf_pad = np.zeros((N_CORES, B_PAD, D), np.float32)
    ni_pad = np.zeros((N_CORES, B_PAD, K), np.int32)
    sf_pad[:, :B_LOCAL] = sf
    ni_pad[:, :B_LOCAL] = ni
    masks = make_masks()
    maps = []
    for c in range(N_CORES):
        # column i of neighbor_idx_t = flat (node, k) slots of instruction i
        nit = ni_pad[c].reshape(NI, P).T
        maps.append(
            {
                "embedding": embedding,
                "self_feats": np.ascontiguousarray(sf_pad[c]),
                "neighbor_idx_t": np.ascontiguousarray(nit),
                "masks": masks,
            }
        )
    return maps


def kernel(embedding, self_feats, neighbor_idx):
    from concourse import bass_utils

    if "nc" not in _cache:
        _cache["nc"] = build_bass()
    nc = _cache["nc"]
    in_maps = make_in_maps(embedding, self_feats, neighbor_idx)
    res = bass_utils.run_bass_kernel_spmd(nc, in_maps, core_ids=list(range(N_CORES)))
    outs = [res.results[c]["out"][:B_LOCAL] for c in range(N_CORES)]
    return np.concatenate(outs, axis=0)

